# revision 16
# baseline (speedup 1.0000x reference)
"""CoreAttention on 8 Trainium2 cores.

Sharding: 32 (batch, head) pairs -> 4 heads per core (cores 0-3: batch 0,
cores 4-7: batch 1). Per core, per head: scores^T = K Q^T in [t, s]
orientation with bf16 operands (fp32 PSUM accumulate), exp on ACT writing
bf16, mask applied as a bitwise AND against a 0xFFFF/0x0000 keep pattern
(int32 view: 2 bf16 lanes per element, so DVE 1x int32 == bf16 2x rate;
the AND form also runs on GPSIMD, which takes the last 4-chunk group of
every block to offload the DVE). Column sums split: chunks 0..11
pair-tree-added on DVE into a [128,2,SBLK] accumulator (wide FD=1024 bf16
adds), chunks 12..15 plus the two accumulator rows contracted on the PE
with a ones vector. P@V as bf16 matmuls accumulating in PSUM,
normalization via reciprocal_approx_fast + gpsimd partition_broadcast +
DVE multiply, double-buffered sums bank so the PE never waits on the
reciprocal read.

Software-pipelined: scores/exp/mask for block i+1 are issued before the
PV/sums/normalize tail of block i. Input DMAs are issued in need-order as
small pieces (32-131KB) so operands land just ahead of their consumers
(one DMA ring moves only ~19GB/s).
Host side only slices/transposes/casts inputs (layout prep).
"""
import sys, math
import numpy as np

sys.path.insert(0, "/opt/trn_rl_repo")

SQ, B, NH, HN = 2048, 2, 16, 128
NCORES = 8
HPC = 4                      # heads per core
TC = SQ // 128               # 16 t-chunks
SBLK = 512                   # s-block width
NSB = SQ // SBLK             # 4 s-blocks
SCALE = 1.0 / math.sqrt(128.0)   # COEFF / NORM_FACTOR = 1/sqrt(hn)
KDVE = 14                        # t-chunks whose column-sums skip the PE
DEPTH = 2                        # software pipeline depth (F blocks ahead of B)

_CACHE = {}


def _build(repeat=1):
    import concourse.bacc as bacc
    import concourse.tile as tile
    from concourse import mybir

    F32, BF16, U32 = mybir.dt.float32, mybir.dt.bfloat16, mybir.dt.uint32
    EXP = mybir.ActivationFunctionType.Exp
    AND = mybir.AluOpType.logical_and

    nc = bacc.Bacc(None, target_bir_lowering=False)
    qT_d = nc.dram_tensor("qT", [HPC, HN, SQ], BF16, kind="ExternalInput")
    kT_d = nc.dram_tensor("kT", [HPC, HN, SQ], BF16, kind="ExternalInput")
    v_d = nc.dram_tensor("v", [HPC, SQ, HN], BF16, kind="ExternalInput")
    # keep swizzled host-side to [p, sb, c, s'] so one s-block is a single
    # DMA with 16KB-contiguous runs per partition (descriptor-efficient)
    keep_d = nc.dram_tensor("keepT", [128, NSB, TC, SBLK], BF16,
                            kind="ExternalInput")
    ctxT_d = nc.dram_tensor("ctxT", [HPC, HN, SQ], BF16, kind="ExternalOutput")

    with tile.TileContext(nc) as tc:
        with (
            tc.tile_pool(name="sbkeep", bufs=1) as sbkeep,
            tc.tile_pool(name="const", bufs=1) as const,
            tc.tile_pool(name="sbqkv", bufs=2) as sbqkv,
            tc.tile_pool(name="sbpt", bufs=DEPTH + 1) as sbpt,
            tc.tile_pool(name="sbacc", bufs=DEPTH + 1) as sbacc,
            tc.tile_pool(name="sbtmp", bufs=2) as sbtmp,
            tc.tile_pool(name="sbe", bufs=3) as sbe,
            tc.tile_pool(name="sbmisc", bufs=2) as sbmisc,
            tc.tile_pool(name="pst", bufs=2, space="PSUM") as pst,
            tc.tile_pool(name="psc", bufs=2, space="PSUM") as psc,
            tc.tile_pool(name="pss", bufs=2, space="PSUM") as pss,
        ):
            keep_t = sbkeep.tile([128, NSB, TC, SBLK], BF16, tag="keep")

            ones_b = const.tile([128, 1], BF16, tag="ob")
            nc.vector.memset(ones_b[:], 1.0)
            warm_src = const.tile([128, SBLK], BF16, tag="warm")
            nc.vector.memset(warm_src[:], 0.0)
            warm_e = const.tile([128, 16], BF16, tag="warme")

            def emit_front(h, sb, qT_t, kT_t):
                """scores -> exp -> mask for (h, sb); returns (pt, acc).

                Mask is a u32 bitwise AND (keep pattern 0xFFFF/0x0000);
                the q=3 group's AND runs on GPSIMD (its chunks feed only
                PE matmuls, so no DVE chain depends on it). Chunks
                0..KDVE-1 are pair-tree-reduced on the DVE into
                acc[128, 2, SBLK]; the PE later contracts chunks
                KDVE..15 plus the two acc rows.
                """
                s0 = sb * SBLK
                pt = sbpt.tile([128, TC, SBLK], BF16, tag="pt")
                acc = sbacc.tile([128, 2, SBLK], BF16, tag="acc")
                tmp = sbtmp.tile([128, 2, SBLK], BF16, tag="tmp")
                for q in range(4):
                    e16 = sbe.tile([128, 4, SBLK], BF16, tag="e")
                    for half in range(2):
                        st = pst.tile([128, 2, SBLK], F32, tag="st")
                        for j in range(2):
                            ti = 4 * q + 2 * half + j
                            nc.tensor.matmul(
                                st[:, j, :],
                                kT_t[:, 128 * ti:128 * (ti + 1)],
                                qT_t[:, s0:s0 + SBLK],
                                start=True, stop=True)
                        nc.scalar.activation(
                            e16[:, 2 * half:2 * half + 2, :], st[:], EXP,
                            scale=SCALE)
                    nc.vector.tensor_mul(
                        pt[:, 4 * q:4 * q + 4, :], e16[:],
                        keep_t[:, sb, 4 * q:4 * q + 4, :])
                    if q == 0:
                        nc.vector.tensor_add(acc[:], pt[:, 0:2, :],
                                             pt[:, 2:4, :])
                    elif q == 1:
                        nc.vector.tensor_add(tmp[:], pt[:, 4:6, :],
                                             pt[:, 6:8, :])
                    elif q == 2:
                        nc.vector.tensor_add(acc[:], acc[:], tmp[:])
                        nc.vector.tensor_add(tmp[:], pt[:, 8:10, :],
                                             pt[:, 10:12, :])
                    else:
                        # late combines on GPSIMD: their inputs are ready
                        # by end-of-front, and the PE consumer (the sums
                        # join matmuls in emit_back) runs two blocks
                        # later, so the strict gpsimd FIFO never blocks
                        nc.gpsimd.tensor_add(acc[:], acc[:], tmp[:])
                        nc.gpsimd.tensor_add(acc[:], acc[:],
                                             pt[:, 12:14, :])
                return pt, acc

            def emit_back(h, sb, pt, acc, v_t, nsplit=1):
                """sums -> PV -> normalize -> store for (h, sb)."""
                s0 = sb * SBLK
                sums_p = pss.tile([1, SBLK], F32, tag="sums")
                for ti in range(KDVE, TC):
                    nc.tensor.matmul(sums_p[:], ones_b[:], pt[:, ti, :],
                                     start=(ti == KDVE), stop=False)
                nc.tensor.matmul(sums_p[:], ones_b[:], acc[:, 0, :],
                                 start=False, stop=False)
                nc.tensor.matmul(sums_p[:], ones_b[:], acc[:, 1, :],
                                 start=False, stop=True)
                ctx_p = psc.tile([128, SBLK], F32, tag="ctx")
                for ti in range(TC):
                    nc.tensor.matmul(ctx_p[:], v_t[:, ti, :], pt[:, ti, :],
                                     start=(ti == 0), stop=(ti == TC - 1))
                w = SBLK // nsplit
                for o in range(0, SBLK, w):
                    recip = sbmisc.tile([1, w], F32, tag="recip")
                    nc.vector.reciprocal_approx_fast(recip[:],
                                                     sums_p[:, o:o + w])
                    rep_s = sbmisc.tile([128, w], F32, tag="reps")
                    nc.gpsimd.partition_broadcast(rep_s[:], recip[:])
                    ctx_s = sbmisc.tile([128, w], BF16, tag="ctxs")
                    nc.vector.tensor_mul(ctx_s[:], ctx_p[:, o:o + w], rep_s[:])
                    nc.sync.dma_start(out=ctxT_d[h, :, s0 + o:s0 + o + w],
                                      in_=ctx_s[:])

            def body(_iv=None):
                # warm the PE clock (HAM) and the ACT exp table with dummy
                # ops that only depend on the memset, while the first DMAs
                # land
                warm_p = pss.tile([1, SBLK], F32, tag="sums")
                for _ in range(6):
                    nc.tensor.matmul(warm_p[:], ones_b[:], warm_src[:],
                                     start=True, stop=True)
                nc.scalar.activation(warm_e[:], warm_src[:, 0:16], EXP,
                                     scale=SCALE)

                pendings = []   # [(h, sb, pt, acc, v_t), ...]
                qkv = {}
                for h in range(HPC):
                    qT_t = sbqkv.tile([128, SQ], BF16, tag="qT")
                    kT_t = sbqkv.tile([128, SQ], BF16, tag="kT")
                    v_t = sbqkv.tile([128, TC, HN], BF16, tag="v")
                    v_r = v_d[h].rearrange("(c p) d -> p c d", p=128)
                    if h == 0:
                        # need-ordered loads, trigger-frugal: each
                        # dma_start costs ~0.6us of SP sequencer time
                        # (DIRECT2D), and consumers wait on whole-DMA
                        # semaphores — so the first-needed operands go
                        # as small pieces and the bulk as few large
                        # descriptor-efficient transfers.
                        for c in range(8):
                            nc.sync.dma_start(
                                out=kT_t[:, 256 * c:256 * (c + 1)],
                                in_=kT_d[h][:, 256 * c:256 * (c + 1)])
                        for c in range(2):
                            nc.sync.dma_start(
                                out=qT_t[:, 256 * c:256 * (c + 1)],
                                in_=qT_d[h][:, 256 * c:256 * (c + 1)])
                        for half in range(2):
                            nc.sync.dma_start(
                                out=keep_t[:, 0, 8 * half:8 * (half + 1), :],
                                in_=keep_d[:, 0, 8 * half:8 * (half + 1), :])
                        for c in range(1, NSB):
                            nc.sync.dma_start(
                                out=qT_t[:, SBLK * c:SBLK * (c + 1)],
                                in_=qT_d[h][:, SBLK * c:SBLK * (c + 1)])
                        nc.sync.dma_start(out=keep_t[:, 1], in_=keep_d[:, 1])
                        for half in range(2):
                            nc.sync.dma_start(
                                out=v_t[:, 8 * half:8 * (half + 1), :],
                                in_=v_r[:, 8 * half:8 * (half + 1), :])
                        nc.sync.dma_start(out=keep_t[:, 2], in_=keep_d[:, 2])
                        nc.sync.dma_start(out=keep_t[:, 3], in_=keep_d[:, 3])
                    else:
                        for half in range(2):
                            cols = slice(SQ // 2 * half, SQ // 2 * (half + 1))
                            nc.sync.dma_start(out=qT_t[:, cols],
                                              in_=qT_d[h][:, cols])
                            nc.sync.dma_start(out=kT_t[:, cols],
                                              in_=kT_d[h][:, cols])
                            nc.sync.dma_start(
                                out=v_t[:, 8 * half:8 * (half + 1), :],
                                in_=v_r[:, 8 * half:8 * (half + 1), :])
                    qkv[h] = (qT_t, kT_t, v_t)
                    last_head = h == HPC - 1
                    for sb in range(NSB):
                        # shallow out the pipeline before the final block
                        # so the drain tail after the last front is short
                        if last_head and sb == NSB - 1:
                            while len(pendings) > 1:
                                emit_back(*pendings.pop(0))
                        pt, acc = emit_front(h, sb, qT_t, kT_t)
                        pendings.append((h, sb, pt, acc, v_t))
                        if len(pendings) > DEPTH:
                            emit_back(*pendings.pop(0))
                for p in pendings:
                    emit_back(*p, nsplit=2)

            if repeat == 1:
                body()
            else:
                with tc.For_i(0, repeat, 1):
                    body()
    nc.compile()
    return nc


def _get_nc(repeat=1):
    if repeat not in _CACHE:
        _CACHE[repeat] = _build(repeat)
    return _CACHE[repeat]


def _make_in_maps(query_layer, key_layer, value_layer, attention_mask):
    import ml_dtypes
    bf16 = ml_dtypes.bfloat16
    q = np.asarray(query_layer, dtype=np.float32)
    k = np.asarray(key_layer, dtype=np.float32)
    v = np.asarray(value_layer, dtype=np.float32)
    m = np.asarray(attention_mask)
    in_maps = []
    for c in range(NCORES):
        b = c // 4
        h0 = 4 * (c % 4)
        hs = slice(h0, h0 + HPC)
        qT = np.ascontiguousarray(
            q[:, b, hs, :].transpose(1, 2, 0)).astype(bf16)    # [4,hn,sq]
        kT = np.ascontiguousarray(
            k[:, b, hs, :].transpose(1, 2, 0)).astype(bf16)
        vv = np.ascontiguousarray(
            v[:, b, hs, :].transpose(1, 0, 2)).astype(bf16)    # [4,sq,hn]
        # keep swizzled to [p, sb, c, s']: keep2[p, sb, c, s'] =
        # keep[t = c*128 + p, s = sb*512 + s']
        keep_ts = (m[b, 0] == 0).T.astype(bf16)                # [t,s] bf16
        keepT = np.ascontiguousarray(
            keep_ts.reshape(16, 128, 4, 512).transpose(1, 2, 0, 3))
        in_maps.append({"qT": qT, "kT": kT, "v": vv, "keepT": keepT})
    return in_maps


def _run(nc, in_maps):
    from concourse.bass_utils import run_bass_kernel_spmd
    return run_bass_kernel_spmd(nc, in_maps, list(range(NCORES)))


def kernel(query_layer, key_layer, value_layer, attention_mask):
    in_maps = _make_in_maps(query_layer, key_layer, value_layer, attention_mask)
    res = _run(_get_nc(1), in_maps)
    out = np.empty((SQ, B, NH, HN), dtype=np.float32)
    for c in range(NCORES):
        b = c // 4
        h0 = 4 * (c % 4)
        ctxT = np.asarray(res.results[c]["ctxT"], dtype=np.float32)   # [4,hn,sq]
        out[:, b, h0:h0 + HPC, :] = ctxT.transpose(2, 0, 1)
    return out.reshape(SQ, B, NH * HN)


# revision 19
# speedup vs baseline: 1.8779x; 1.8779x over previous
"""CoreAttention on 8 Trainium2 cores.

Sharding: 32 (batch, head) pairs -> 4 heads per core (cores 0-3: batch 0,
cores 4-7: batch 1). Per core, per head: scores^T = K Q^T in [t, s]
orientation with bf16 operands (fp32 PSUM accumulate), exp on ACT writing
bf16, mask applied as a bitwise AND against a 0xFFFF/0x0000 keep pattern
(int32 view: 2 bf16 lanes per element, so DVE 1x int32 == bf16 2x rate;
the AND form also runs on GPSIMD, which takes the last 4-chunk group of
every block to offload the DVE). Column sums split: chunks 0..11
pair-tree-added on DVE into a [128,2,SBLK] accumulator (wide FD=1024 bf16
adds), chunks 12..15 plus the two accumulator rows contracted on the PE
with a ones vector. P@V as bf16 matmuls accumulating in PSUM,
normalization via reciprocal_approx_fast + gpsimd partition_broadcast +
DVE multiply, double-buffered sums bank so the PE never waits on the
reciprocal read.

Software-pipelined: scores/exp/mask for block i+1 are issued before the
PV/sums/normalize tail of block i. Input DMAs are issued in need-order as
small pieces (32-131KB) so operands land just ahead of their consumers
(one DMA ring moves only ~19GB/s).
Host side only slices/transposes/casts inputs (layout prep).
"""
import sys, math
import numpy as np

sys.path.insert(0, "/opt/trn_rl_repo")

SQ, B, NH, HN = 2048, 2, 16, 128
NCORES = 8
HPC = 4                      # heads per core
TC = SQ // 128               # 16 t-chunks
SBLK = 512                   # s-block width
NSB = SQ // SBLK             # 4 s-blocks
SCALE = 1.0 / math.sqrt(128.0)   # COEFF / NORM_FACTOR = 1/sqrt(hn)
KDVE = 14                        # t-chunks whose column-sums skip the PE
DEPTH = 2                        # software pipeline depth (F blocks ahead of B)

_CACHE = {}


def _build(repeat=1):
    import concourse.bacc as bacc
    import concourse.tile as tile
    from concourse import mybir

    F32, BF16, U32 = mybir.dt.float32, mybir.dt.bfloat16, mybir.dt.uint32
    EXP = mybir.ActivationFunctionType.Exp
    AND = mybir.AluOpType.logical_and

    nc = bacc.Bacc(None, target_bir_lowering=False)
    qT_d = nc.dram_tensor("qT", [HPC, HN, SQ], BF16, kind="ExternalInput")
    kT_d = nc.dram_tensor("kT", [HPC, HN, SQ], BF16, kind="ExternalInput")
    v_d = nc.dram_tensor("v", [HPC, SQ, HN], BF16, kind="ExternalInput")
    # keep swizzled host-side to [p, sb, c, s'] so one s-block is a single
    # DMA with 16KB-contiguous runs per partition (descriptor-efficient)
    keep_d = nc.dram_tensor("keepT", [128, NSB, TC, SBLK], BF16,
                            kind="ExternalInput")
    ctxT_d = nc.dram_tensor("ctxT", [HPC, HN, SQ], BF16, kind="ExternalOutput")

    with tile.TileContext(nc) as tc:
        with (
            tc.tile_pool(name="sbkeep", bufs=1) as sbkeep,
            tc.tile_pool(name="const", bufs=1) as const,
            tc.tile_pool(name="sbqkv", bufs=2) as sbqkv,
            tc.tile_pool(name="sbpt", bufs=DEPTH + 1) as sbpt,
            tc.tile_pool(name="sbacc", bufs=DEPTH + 1) as sbacc,
            tc.tile_pool(name="sbtmp", bufs=2) as sbtmp,
            tc.tile_pool(name="sbe", bufs=3) as sbe,
            tc.tile_pool(name="sbmisc", bufs=2) as sbmisc,
            tc.tile_pool(name="pst", bufs=2, space="PSUM") as pst,
            tc.tile_pool(name="psc", bufs=2, space="PSUM") as psc,
            tc.tile_pool(name="pss", bufs=2, space="PSUM") as pss,
        ):
            keep_t = sbkeep.tile([128, NSB, TC, SBLK], BF16, tag="keep")

            ones_b = const.tile([128, 1], BF16, tag="ob")
            nc.vector.memset(ones_b[:], 1.0)
            # full ones stationary: the sums joins then produce the column
            # sums replicated across all 128 partitions (same N=512
            # streaming cost as an M=1 join), so no partition_broadcast
            # is ever needed for the normalize
            ones_m = const.tile([128, 128], BF16, tag="om")
            nc.vector.memset(ones_m[:], 1.0)
            warm_src = const.tile([128, SBLK], BF16, tag="warm")
            nc.vector.memset(warm_src[:], 0.0)
            warm_e = const.tile([128, 16], BF16, tag="warme")

            def emit_front(h, sb, qT_t, kT_t):
                """scores -> exp -> mask for (h, sb); returns (pt, acc).

                Mask is a u32 bitwise AND (keep pattern 0xFFFF/0x0000);
                the q=3 group's AND runs on GPSIMD (its chunks feed only
                PE matmuls, so no DVE chain depends on it). Chunks
                0..KDVE-1 are pair-tree-reduced on the DVE into
                acc[128, 2, SBLK]; the PE later contracts chunks
                KDVE..15 plus the two acc rows.
                """
                s0 = sb * SBLK
                pt = sbpt.tile([128, TC, SBLK], BF16, tag="pt")
                acc = sbacc.tile([128, 2, SBLK], BF16, tag="acc")
                tmp = sbtmp.tile([128, 2, SBLK], BF16, tag="tmp")
                for q in range(4):
                    e16 = sbe.tile([128, 4, SBLK], BF16, tag="e")
                    for half in range(2):
                        st = pst.tile([128, 2, SBLK], F32, tag="st")
                        for j in range(2):
                            ti = 4 * q + 2 * half + j
                            nc.tensor.matmul(
                                st[:, j, :],
                                kT_t[:, 128 * ti:128 * (ti + 1)],
                                qT_t[:, s0:s0 + SBLK],
                                start=True, stop=True)
                        nc.scalar.activation(
                            e16[:, 2 * half:2 * half + 2, :], st[:], EXP,
                            scale=SCALE)
                    nc.vector.tensor_mul(
                        pt[:, 4 * q:4 * q + 4, :], e16[:],
                        keep_t[:, sb, 4 * q:4 * q + 4, :])
                    if q == 0:
                        nc.vector.tensor_add(acc[:], pt[:, 0:2, :],
                                             pt[:, 2:4, :])
                    elif q == 1:
                        nc.vector.tensor_add(tmp[:], pt[:, 4:6, :],
                                             pt[:, 6:8, :])
                    elif q == 2:
                        nc.vector.tensor_add(acc[:], acc[:], tmp[:])
                        nc.vector.tensor_add(tmp[:], pt[:, 8:10, :],
                                             pt[:, 10:12, :])
                    else:
                        # late combines on GPSIMD: their inputs are ready
                        # by end-of-front, and the PE consumer (the sums
                        # join matmuls in emit_back) runs two blocks
                        # later, so the strict gpsimd FIFO never blocks
                        nc.gpsimd.tensor_add(acc[:], acc[:], tmp[:])
                        nc.gpsimd.tensor_add(acc[:], acc[:],
                                             pt[:, 12:14, :])
                return pt, acc

            def emit_back(h, sb, pt, acc, v_t, nsplit=1):
                """sums -> PV -> normalize -> store for (h, sb)."""
                s0 = sb * SBLK
                sums_p = pss.tile([128, SBLK], F32, tag="sums")
                for ti in range(KDVE, TC):
                    nc.tensor.matmul(sums_p[:], ones_m[:], pt[:, ti, :],
                                     start=(ti == KDVE), stop=False)
                nc.tensor.matmul(sums_p[:], ones_m[:], acc[:, 0, :],
                                 start=False, stop=False)
                nc.tensor.matmul(sums_p[:], ones_m[:], acc[:, 1, :],
                                 start=False, stop=True)
                ctx_p = psc.tile([128, SBLK], F32, tag="ctx")
                for ti in range(TC):
                    nc.tensor.matmul(ctx_p[:], v_t[:, ti, :], pt[:, ti, :],
                                     start=(ti == 0), stop=(ti == TC - 1))
                w = SBLK // nsplit
                for o in range(0, SBLK, w):
                    recip = sbmisc.tile([128, w], F32, tag="recip")
                    nc.vector.reciprocal_approx_fast(recip[:],
                                                     sums_p[:, o:o + w])
                    ctx_s = sbmisc.tile([128, w], BF16, tag="ctxs")
                    nc.vector.tensor_mul(ctx_s[:], ctx_p[:, o:o + w], recip[:])
                    nc.sync.dma_start(out=ctxT_d[h, :, s0 + o:s0 + o + w],
                                      in_=ctx_s[:])

            def body(_iv=None):
                # warm the PE clock (HAM) and the ACT exp table with dummy
                # ops that only depend on the memset, while the first DMAs
                # land
                warm_p = pss.tile([1, SBLK], F32, tag="sums")
                for _ in range(6):
                    nc.tensor.matmul(warm_p[:], ones_b[:], warm_src[:],
                                     start=True, stop=True)
                nc.scalar.activation(warm_e[:], warm_src[:, 0:16], EXP,
                                     scale=SCALE)

                pendings = []   # [(h, sb, pt, acc, v_t), ...]
                qkv = {}
                for h in range(HPC):
                    qT_t = sbqkv.tile([128, SQ], BF16, tag="qT")
                    kT_t = sbqkv.tile([128, SQ], BF16, tag="kT")
                    v_t = sbqkv.tile([128, TC, HN], BF16, tag="v")
                    v_r = v_d[h].rearrange("(c p) d -> p c d", p=128)
                    if h == 0:
                        # need-ordered loads, trigger-frugal: each
                        # dma_start costs ~0.6us of SP sequencer time
                        # (DIRECT2D), and consumers wait on whole-DMA
                        # semaphores — so the first-needed operands go
                        # as small pieces and the bulk as few large
                        # descriptor-efficient transfers.
                        for c in range(8):
                            nc.sync.dma_start(
                                out=kT_t[:, 256 * c:256 * (c + 1)],
                                in_=kT_d[h][:, 256 * c:256 * (c + 1)])
                        for c in range(2):
                            nc.sync.dma_start(
                                out=qT_t[:, 256 * c:256 * (c + 1)],
                                in_=qT_d[h][:, 256 * c:256 * (c + 1)])
                        for half in range(2):
                            nc.sync.dma_start(
                                out=keep_t[:, 0, 8 * half:8 * (half + 1), :],
                                in_=keep_d[:, 0, 8 * half:8 * (half + 1), :])
                        for c in range(1, NSB):
                            nc.sync.dma_start(
                                out=qT_t[:, SBLK * c:SBLK * (c + 1)],
                                in_=qT_d[h][:, SBLK * c:SBLK * (c + 1)])
                        nc.sync.dma_start(out=keep_t[:, 1], in_=keep_d[:, 1])
                        for half in range(2):
                            nc.sync.dma_start(
                                out=v_t[:, 8 * half:8 * (half + 1), :],
                                in_=v_r[:, 8 * half:8 * (half + 1), :])
                        nc.sync.dma_start(out=keep_t[:, 2], in_=keep_d[:, 2])
                        nc.sync.dma_start(out=keep_t[:, 3], in_=keep_d[:, 3])
                    else:
                        for half in range(2):
                            cols = slice(SQ // 2 * half, SQ // 2 * (half + 1))
                            nc.sync.dma_start(out=qT_t[:, cols],
                                              in_=qT_d[h][:, cols])
                            nc.sync.dma_start(out=kT_t[:, cols],
                                              in_=kT_d[h][:, cols])
                            nc.sync.dma_start(
                                out=v_t[:, 8 * half:8 * (half + 1), :],
                                in_=v_r[:, 8 * half:8 * (half + 1), :])
                    qkv[h] = (qT_t, kT_t, v_t)
                    last_head = h == HPC - 1
                    for sb in range(NSB):
                        # shallow out the pipeline before the final block
                        # so the drain tail after the last front is short
                        if last_head and sb == NSB - 1:
                            while len(pendings) > 1:
                                emit_back(*pendings.pop(0))
                        pt, acc = emit_front(h, sb, qT_t, kT_t)
                        pendings.append((h, sb, pt, acc, v_t))
                        if len(pendings) > DEPTH:
                            emit_back(*pendings.pop(0))
                for i, p in enumerate(pendings):
                    emit_back(*p, nsplit=1 + i)

            if repeat == 1:
                body()
            else:
                with tc.For_i(0, repeat, 1):
                    body()
    nc.compile()
    return nc


def _get_nc(repeat=1):
    if repeat not in _CACHE:
        _CACHE[repeat] = _build(repeat)
    return _CACHE[repeat]


def _make_in_maps(query_layer, key_layer, value_layer, attention_mask):
    import ml_dtypes
    bf16 = ml_dtypes.bfloat16
    q = np.asarray(query_layer, dtype=np.float32)
    k = np.asarray(key_layer, dtype=np.float32)
    v = np.asarray(value_layer, dtype=np.float32)
    m = np.asarray(attention_mask)
    in_maps = []
    for c in range(NCORES):
        b = c // 4
        h0 = 4 * (c % 4)
        hs = slice(h0, h0 + HPC)
        qT = np.ascontiguousarray(
            q[:, b, hs, :].transpose(1, 2, 0)).astype(bf16)    # [4,hn,sq]
        kT = np.ascontiguousarray(
            k[:, b, hs, :].transpose(1, 2, 0)).astype(bf16)
        vv = np.ascontiguousarray(
            v[:, b, hs, :].transpose(1, 0, 2)).astype(bf16)    # [4,sq,hn]
        # keep swizzled to [p, sb, c, s']: keep2[p, sb, c, s'] =
        # keep[t = c*128 + p, s = sb*512 + s']
        keep_ts = (m[b, 0] == 0).T.astype(bf16)                # [t,s] bf16
        keepT = np.ascontiguousarray(
            keep_ts.reshape(16, 128, 4, 512).transpose(1, 2, 0, 3))
        in_maps.append({"qT": qT, "kT": kT, "v": vv, "keepT": keepT})
    return in_maps


def _run(nc, in_maps):
    from concourse.bass_utils import run_bass_kernel_spmd
    return run_bass_kernel_spmd(nc, in_maps, list(range(NCORES)))


def kernel(query_layer, key_layer, value_layer, attention_mask):
    in_maps = _make_in_maps(query_layer, key_layer, value_layer, attention_mask)
    res = _run(_get_nc(1), in_maps)
    out = np.empty((SQ, B, NH, HN), dtype=np.float32)
    for c in range(NCORES):
        b = c // 4
        h0 = 4 * (c % 4)
        ctxT = np.asarray(res.results[c]["ctxT"], dtype=np.float32)   # [4,hn,sq]
        out[:, b, h0:h0 + HPC, :] = ctxT.transpose(2, 0, 1)
    return out.reshape(SQ, B, NH * HN)


# revision 21
# speedup vs baseline: 2.3978x; 1.2769x over previous
"""CoreAttention on 8 Trainium2 cores.

Sharding: 32 (batch, head) pairs -> 4 heads per core (cores 0-3: batch 0,
cores 4-7: batch 1). Per core, per head: scores^T = K Q^T in [t, s]
orientation with bf16 operands (fp32 PSUM accumulate), exp on ACT writing
bf16, mask applied as a bitwise AND against a 0xFFFF/0x0000 keep pattern
(int32 view: 2 bf16 lanes per element, so DVE 1x int32 == bf16 2x rate;
the AND form also runs on GPSIMD, which takes the last 4-chunk group of
every block to offload the DVE). Column sums split: chunks 0..11
pair-tree-added on DVE into a [128,2,SBLK] accumulator (wide FD=1024 bf16
adds), chunks 12..15 plus the two accumulator rows contracted on the PE
with a ones vector. P@V as bf16 matmuls accumulating in PSUM,
normalization via reciprocal_approx_fast + gpsimd partition_broadcast +
DVE multiply, double-buffered sums bank so the PE never waits on the
reciprocal read.

Software-pipelined: scores/exp/mask for block i+1 are issued before the
PV/sums/normalize tail of block i. Input DMAs are issued in need-order as
small pieces (32-131KB) so operands land just ahead of their consumers
(one DMA ring moves only ~19GB/s).
Host side only slices/transposes/casts inputs (layout prep).
"""
import sys, math
import numpy as np

sys.path.insert(0, "/opt/trn_rl_repo")

SQ, B, NH, HN = 2048, 2, 16, 128
NCORES = 8
HPC = 4                      # heads per core
TC = SQ // 128               # 16 t-chunks
SBLK = 512                   # s-block width
NSB = SQ // SBLK             # 4 s-blocks
SCALE = 1.0 / math.sqrt(128.0)   # COEFF / NORM_FACTOR = 1/sqrt(hn)
KDVE = 12                        # t-chunks whose column-sums go via DVE adds
DEPTH = 2                        # software pipeline depth (F blocks ahead of B)

_CACHE = {}


def _build(repeat=1):
    import concourse.bacc as bacc
    import concourse.tile as tile
    from concourse import mybir

    F32, BF16, U32 = mybir.dt.float32, mybir.dt.bfloat16, mybir.dt.uint32
    EXP = mybir.ActivationFunctionType.Exp
    AND = mybir.AluOpType.logical_and

    nc = bacc.Bacc(None, target_bir_lowering=False)
    qT_d = nc.dram_tensor("qT", [HPC, HN, SQ], BF16, kind="ExternalInput")
    kT_d = nc.dram_tensor("kT", [HPC, HN, SQ], BF16, kind="ExternalInput")
    v_d = nc.dram_tensor("v", [HPC, SQ, HN], BF16, kind="ExternalInput")
    # keep swizzled host-side to [p, sb, c, s'] so one s-block is a single
    # DMA with 16KB-contiguous runs per partition (descriptor-efficient)
    keep_d = nc.dram_tensor("keepT", [128, NSB, TC, SBLK], BF16,
                            kind="ExternalInput")
    ctxT_d = nc.dram_tensor("ctxT", [HPC, HN, SQ], BF16, kind="ExternalOutput")

    with tile.TileContext(nc) as tc:
        with (
            tc.tile_pool(name="sbkeep", bufs=1) as sbkeep,
            tc.tile_pool(name="const", bufs=1) as const,
            tc.tile_pool(name="sbqkv", bufs=2) as sbqkv,
            tc.tile_pool(name="sbpt", bufs=DEPTH + 1) as sbpt,
            tc.tile_pool(name="sbacc", bufs=DEPTH + 1) as sbacc,
            tc.tile_pool(name="sbtmp", bufs=2) as sbtmp,
            tc.tile_pool(name="sbe", bufs=3) as sbe,
            tc.tile_pool(name="sbmisc", bufs=2) as sbmisc,
            tc.tile_pool(name="pst", bufs=2, space="PSUM") as pst,
            tc.tile_pool(name="psc", bufs=2, space="PSUM") as psc,
            tc.tile_pool(name="pss", bufs=2, space="PSUM") as pss,
        ):
            keep_t = sbkeep.tile([128, NSB, TC, SBLK], BF16, tag="keep")

            ones_b = const.tile([128, 1], BF16, tag="ob")
            nc.vector.memset(ones_b[:], 1.0)
            # full ones stationary: the sums joins then produce the column
            # sums replicated across all 128 partitions (same N=512
            # streaming cost as an M=1 join), so no partition_broadcast
            # is ever needed for the normalize
            ones_m = const.tile([128, 128], BF16, tag="om")
            nc.vector.memset(ones_m[:], 1.0)
            warm_src = const.tile([128, SBLK], BF16, tag="warm")
            nc.vector.memset(warm_src[:], 0.0)
            warm_e = const.tile([128, 16], BF16, tag="warme")

            def emit_front(h, sb, qT_t, kT_t):
                """scores -> exp -> mask for (h, sb); returns (pt, acc).

                Mask is a u32 bitwise AND (keep pattern 0xFFFF/0x0000);
                the q=3 group's AND runs on GPSIMD (its chunks feed only
                PE matmuls, so no DVE chain depends on it). Chunks
                0..KDVE-1 are pair-tree-reduced on the DVE into
                acc[128, 2, SBLK]; the PE later contracts chunks
                KDVE..15 plus the two acc rows.
                """
                s0 = sb * SBLK
                pt = sbpt.tile([128, TC, SBLK], BF16, tag="pt")
                acc = sbacc.tile([128, 2, SBLK], BF16, tag="acc")
                tmp = sbtmp.tile([128, 2, SBLK], BF16, tag="tmp")
                for q in range(4):
                    e16 = sbe.tile([128, 4, SBLK], BF16, tag="e")
                    for half in range(2):
                        st = pst.tile([128, 2, SBLK], F32, tag="st")
                        for j in range(2):
                            ti = 4 * q + 2 * half + j
                            nc.tensor.matmul(
                                st[:, j, :],
                                kT_t[:, 128 * ti:128 * (ti + 1)],
                                qT_t[:, s0:s0 + SBLK],
                                start=True, stop=True)
                        nc.scalar.activation(
                            e16[:, 2 * half:2 * half + 2, :], st[:], EXP,
                            scale=SCALE)
                    nc.vector.tensor_mul(
                        pt[:, 4 * q:4 * q + 4, :], e16[:],
                        keep_t[:, sb, 4 * q:4 * q + 4, :])
                    if q == 0:
                        nc.vector.tensor_add(acc[:], pt[:, 0:2, :],
                                             pt[:, 2:4, :])
                    elif q == 1:
                        nc.vector.tensor_add(tmp[:], pt[:, 4:6, :],
                                             pt[:, 6:8, :])
                    elif q == 2:
                        nc.vector.tensor_add(acc[:], acc[:], tmp[:])
                        nc.vector.tensor_add(tmp[:], pt[:, 8:10, :],
                                             pt[:, 10:12, :])
                    else:
                        # NOTE: gpsimd must not take any of this work —
                        # it shares an SBUF port with the DVE, and heavy
                        # gpsimd elementwise traffic slows every DVE op
                        # by ~20% (measured)
                        nc.vector.tensor_add(acc[:], acc[:], tmp[:])
                return pt, acc

            def emit_back(h, sb, pt, acc, v_t, nsplit=1):
                """sums -> PV -> normalize -> store for (h, sb)."""
                s0 = sb * SBLK
                sums_p = pss.tile([128, SBLK], F32, tag="sums")
                for ti in range(KDVE, TC):
                    nc.tensor.matmul(sums_p[:], ones_m[:], pt[:, ti, :],
                                     start=(ti == KDVE), stop=False)
                nc.tensor.matmul(sums_p[:], ones_m[:], acc[:, 0, :],
                                 start=False, stop=False)
                nc.tensor.matmul(sums_p[:], ones_m[:], acc[:, 1, :],
                                 start=False, stop=True)
                ctx_p = psc.tile([128, SBLK], F32, tag="ctx")
                for ti in range(TC):
                    nc.tensor.matmul(ctx_p[:], v_t[:, ti, :], pt[:, ti, :],
                                     start=(ti == 0), stop=(ti == TC - 1))
                w = SBLK // nsplit
                for o in range(0, SBLK, w):
                    recip = sbmisc.tile([128, w], F32, tag="recip")
                    nc.vector.reciprocal_approx_fast(recip[:],
                                                     sums_p[:, o:o + w])
                    ctx_s = sbmisc.tile([128, w], BF16, tag="ctxs")
                    nc.vector.tensor_mul(ctx_s[:], ctx_p[:, o:o + w], recip[:])
                    nc.sync.dma_start(out=ctxT_d[h, :, s0 + o:s0 + o + w],
                                      in_=ctx_s[:])

            def body(_iv=None):
                # warm the PE clock (HAM) and the ACT exp table with dummy
                # ops that only depend on the memset, while the first DMAs
                # land
                warm_p = pss.tile([1, SBLK], F32, tag="sums")
                for _ in range(6):
                    nc.tensor.matmul(warm_p[:], ones_b[:], warm_src[:],
                                     start=True, stop=True)
                nc.scalar.activation(warm_e[:], warm_src[:, 0:16], EXP,
                                     scale=SCALE)

                pendings = []   # [(h, sb, pt, acc, v_t), ...]
                qkv = {}
                for h in range(HPC):
                    qT_t = sbqkv.tile([128, SQ], BF16, tag="qT")
                    kT_t = sbqkv.tile([128, SQ], BF16, tag="kT")
                    v_t = sbqkv.tile([128, TC, HN], BF16, tag="v")
                    v_r = v_d[h].rearrange("(c p) d -> p c d", p=128)
                    if h == 0:
                        # need-ordered loads, trigger-frugal: each
                        # dma_start costs ~0.6us of SP sequencer time
                        # (DIRECT2D), and consumers wait on whole-DMA
                        # semaphores — so the first-needed operands go
                        # as small pieces and the bulk as few large
                        # descriptor-efficient transfers.
                        for c in range(8):
                            nc.sync.dma_start(
                                out=kT_t[:, 256 * c:256 * (c + 1)],
                                in_=kT_d[h][:, 256 * c:256 * (c + 1)])
                        for c in range(2):
                            nc.sync.dma_start(
                                out=qT_t[:, 256 * c:256 * (c + 1)],
                                in_=qT_d[h][:, 256 * c:256 * (c + 1)])
                        for half in range(2):
                            nc.sync.dma_start(
                                out=keep_t[:, 0, 8 * half:8 * (half + 1), :],
                                in_=keep_d[:, 0, 8 * half:8 * (half + 1), :])
                        for c in range(1, NSB):
                            nc.sync.dma_start(
                                out=qT_t[:, SBLK * c:SBLK * (c + 1)],
                                in_=qT_d[h][:, SBLK * c:SBLK * (c + 1)])
                        nc.sync.dma_start(out=keep_t[:, 1], in_=keep_d[:, 1])
                        for half in range(2):
                            nc.sync.dma_start(
                                out=v_t[:, 8 * half:8 * (half + 1), :],
                                in_=v_r[:, 8 * half:8 * (half + 1), :])
                        nc.sync.dma_start(out=keep_t[:, 2], in_=keep_d[:, 2])
                        nc.sync.dma_start(out=keep_t[:, 3], in_=keep_d[:, 3])
                    else:
                        for half in range(2):
                            cols = slice(SQ // 2 * half, SQ // 2 * (half + 1))
                            nc.sync.dma_start(out=qT_t[:, cols],
                                              in_=qT_d[h][:, cols])
                            nc.sync.dma_start(out=kT_t[:, cols],
                                              in_=kT_d[h][:, cols])
                            nc.sync.dma_start(
                                out=v_t[:, 8 * half:8 * (half + 1), :],
                                in_=v_r[:, 8 * half:8 * (half + 1), :])
                    qkv[h] = (qT_t, kT_t, v_t)
                    last_head = h == HPC - 1
                    for sb in range(NSB):
                        # shallow out the pipeline before the final block
                        # so the drain tail after the last front is short
                        if last_head and sb == NSB - 1:
                            while len(pendings) > 1:
                                emit_back(*pendings.pop(0))
                        pt, acc = emit_front(h, sb, qT_t, kT_t)
                        pendings.append((h, sb, pt, acc, v_t))
                        if len(pendings) > DEPTH:
                            emit_back(*pendings.pop(0))
                for i, p in enumerate(pendings):
                    emit_back(*p, nsplit=1 + i)

            if repeat == 1:
                body()
            else:
                with tc.For_i(0, repeat, 1):
                    body()
    nc.compile()
    return nc


def _get_nc(repeat=1):
    if repeat not in _CACHE:
        _CACHE[repeat] = _build(repeat)
    return _CACHE[repeat]


def _make_in_maps(query_layer, key_layer, value_layer, attention_mask):
    import ml_dtypes
    bf16 = ml_dtypes.bfloat16
    q = np.asarray(query_layer, dtype=np.float32)
    k = np.asarray(key_layer, dtype=np.float32)
    v = np.asarray(value_layer, dtype=np.float32)
    m = np.asarray(attention_mask)
    in_maps = []
    for c in range(NCORES):
        b = c // 4
        h0 = 4 * (c % 4)
        hs = slice(h0, h0 + HPC)
        qT = np.ascontiguousarray(
            q[:, b, hs, :].transpose(1, 2, 0)).astype(bf16)    # [4,hn,sq]
        kT = np.ascontiguousarray(
            k[:, b, hs, :].transpose(1, 2, 0)).astype(bf16)
        vv = np.ascontiguousarray(
            v[:, b, hs, :].transpose(1, 0, 2)).astype(bf16)    # [4,sq,hn]
        # keep swizzled to [p, sb, c, s']: keep2[p, sb, c, s'] =
        # keep[t = c*128 + p, s = sb*512 + s']
        keep_ts = (m[b, 0] == 0).T.astype(bf16)                # [t,s] bf16
        keepT = np.ascontiguousarray(
            keep_ts.reshape(16, 128, 4, 512).transpose(1, 2, 0, 3))
        in_maps.append({"qT": qT, "kT": kT, "v": vv, "keepT": keepT})
    return in_maps


def _run(nc, in_maps):
    from concourse.bass_utils import run_bass_kernel_spmd
    return run_bass_kernel_spmd(nc, in_maps, list(range(NCORES)))


def kernel(query_layer, key_layer, value_layer, attention_mask):
    in_maps = _make_in_maps(query_layer, key_layer, value_layer, attention_mask)
    res = _run(_get_nc(1), in_maps)
    out = np.empty((SQ, B, NH, HN), dtype=np.float32)
    for c in range(NCORES):
        b = c // 4
        h0 = 4 * (c % 4)
        ctxT = np.asarray(res.results[c]["ctxT"], dtype=np.float32)   # [4,hn,sq]
        out[:, b, h0:h0 + HPC, :] = ctxT.transpose(2, 0, 1)
    return out.reshape(SQ, B, NH * HN)


# revision 25
# speedup vs baseline: 2.4501x; 1.0218x over previous
"""CoreAttention on 8 Trainium2 cores.

Sharding: 32 (batch, head) pairs -> 4 heads per core (cores 0-3: batch 0,
cores 4-7: batch 1). Per core, per head: scores^T = K Q^T in [t, s]
orientation with bf16 operands (fp32 PSUM accumulate), exp on ACT writing
bf16, mask applied as a bitwise AND against a 0xFFFF/0x0000 keep pattern
(int32 view: 2 bf16 lanes per element, so DVE 1x int32 == bf16 2x rate;
the AND form also runs on GPSIMD, which takes the last 4-chunk group of
every block to offload the DVE). Column sums split: chunks 0..11
pair-tree-added on DVE into a [128,2,SBLK] accumulator (wide FD=1024 bf16
adds), chunks 12..15 plus the two accumulator rows contracted on the PE
with a ones vector. P@V as bf16 matmuls accumulating in PSUM,
normalization via reciprocal_approx_fast + gpsimd partition_broadcast +
DVE multiply, double-buffered sums bank so the PE never waits on the
reciprocal read.

Software-pipelined: scores/exp/mask for block i+1 are issued before the
PV/sums/normalize tail of block i. Input DMAs are issued in need-order as
small pieces (32-131KB) so operands land just ahead of their consumers
(one DMA ring moves only ~19GB/s).
Host side only slices/transposes/casts inputs (layout prep).
"""
import sys, math
import numpy as np

sys.path.insert(0, "/opt/trn_rl_repo")

SQ, B, NH, HN = 2048, 2, 16, 128
NCORES = 8
HPC = 4                      # heads per core
TC = SQ // 128               # 16 t-chunks
SBLK = 512                   # s-block width
NSB = SQ // SBLK             # 4 s-blocks
SCALE = 1.0 / math.sqrt(128.0)   # COEFF / NORM_FACTOR = 1/sqrt(hn)
KDVE = 12                        # t-chunks whose column-sums go via DVE adds
DEPTH = 2                        # software pipeline depth (F blocks ahead of B)

_CACHE = {}


def _build(repeat=1):
    import concourse.bacc as bacc
    import concourse.tile as tile
    from concourse import mybir

    F32, BF16, U32 = mybir.dt.float32, mybir.dt.bfloat16, mybir.dt.uint32
    EXP = mybir.ActivationFunctionType.Exp
    AND = mybir.AluOpType.logical_and

    nc = bacc.Bacc(None, target_bir_lowering=False)
    qT_d = nc.dram_tensor("qT", [HPC, HN, SQ], BF16, kind="ExternalInput")
    kT_d = nc.dram_tensor("kT", [HPC, HN, SQ], BF16, kind="ExternalInput")
    v_d = nc.dram_tensor("v", [HPC, SQ, HN], BF16, kind="ExternalInput")
    # keep swizzled host-side to [p, sb, c, s'] so one s-block is a single
    # DMA with 16KB-contiguous runs per partition (descriptor-efficient)
    keep_d = nc.dram_tensor("keepT", [128, NSB, TC, SBLK], BF16,
                            kind="ExternalInput")
    ctxT_d = nc.dram_tensor("ctxT", [HPC, HN, SQ], BF16, kind="ExternalOutput")

    with tile.TileContext(nc) as tc:
        with (
            tc.tile_pool(name="sbkeep", bufs=1) as sbkeep,
            tc.tile_pool(name="const", bufs=1) as const,
            tc.tile_pool(name="sbqkv", bufs=2) as sbqkv,
            tc.tile_pool(name="sbpt", bufs=DEPTH + 2) as sbpt,
            tc.tile_pool(name="sbacc", bufs=DEPTH + 2) as sbacc,
            tc.tile_pool(name="sbtmp", bufs=2) as sbtmp,
            tc.tile_pool(name="sbe", bufs=4) as sbe,
            tc.tile_pool(name="sbmisc", bufs=2) as sbmisc,
            tc.tile_pool(name="pst", bufs=2, space="PSUM") as pst,
            tc.tile_pool(name="psc", bufs=2, space="PSUM") as psc,
            tc.tile_pool(name="pss", bufs=2, space="PSUM") as pss,
        ):
            keep_t = sbkeep.tile([128, NSB, TC, SBLK], BF16, tag="keep")

            ones_b = const.tile([128, 1], BF16, tag="ob")
            nc.vector.memset(ones_b[:], 1.0)
            # full ones stationary: the sums joins then produce the column
            # sums replicated across all 128 partitions (same N=512
            # streaming cost as an M=1 join), so no partition_broadcast
            # is ever needed for the normalize
            ones_m = const.tile([128, 128], BF16, tag="om")
            nc.vector.memset(ones_m[:], 1.0)
            warm_src = const.tile([128, SBLK], BF16, tag="warm")
            nc.vector.memset(warm_src[:], 0.0)
            warm_e = const.tile([128, 16], BF16, tag="warme")

            def emit_front(h, sb, qT_t, kT_t):
                """scores -> exp -> mask for (h, sb); returns (pt, acc).

                Mask is a u32 bitwise AND (keep pattern 0xFFFF/0x0000);
                the q=3 group's AND runs on GPSIMD (its chunks feed only
                PE matmuls, so no DVE chain depends on it). Chunks
                0..KDVE-1 are pair-tree-reduced on the DVE into
                acc[128, 2, SBLK]; the PE later contracts chunks
                KDVE..15 plus the two acc rows.
                """
                s0 = sb * SBLK
                pt = sbpt.tile([128, TC, SBLK], BF16, tag="pt")
                acc = sbacc.tile([128, 2, SBLK], BF16, tag="acc")
                tmp = sbtmp.tile([128, 2, SBLK], BF16, tag="tmp")
                for q in range(4):
                    e16 = sbe.tile([128, 4, SBLK], BF16, tag="e")
                    for half in range(2):
                        st = pst.tile([128, 2, SBLK], F32, tag="st")
                        for j in range(2):
                            ti = 4 * q + 2 * half + j
                            nc.tensor.matmul(
                                st[:, j, :],
                                kT_t[:, 128 * ti:128 * (ti + 1)],
                                qT_t[:, s0:s0 + SBLK],
                                start=True, stop=True)
                        nc.scalar.activation(
                            e16[:, 2 * half:2 * half + 2, :], st[:], EXP,
                            scale=SCALE)
                    nc.vector.tensor_mul(
                        pt[:, 4 * q:4 * q + 4, :], e16[:],
                        keep_t[:, sb, 4 * q:4 * q + 4, :])
                    if q == 0:
                        nc.vector.tensor_add(acc[:], pt[:, 0:2, :],
                                             pt[:, 2:4, :])
                    elif q == 1:
                        nc.vector.tensor_add(tmp[:], pt[:, 4:6, :],
                                             pt[:, 6:8, :])
                    elif q == 2:
                        nc.vector.tensor_add(acc[:], acc[:], tmp[:])
                        nc.vector.tensor_add(tmp[:], pt[:, 8:10, :],
                                             pt[:, 10:12, :])
                    else:
                        # NOTE: gpsimd must not take any of this work —
                        # it shares an SBUF port with the DVE, and heavy
                        # gpsimd elementwise traffic slows every DVE op
                        # by ~20% (measured)
                        nc.vector.tensor_add(acc[:], acc[:], tmp[:])
                return pt, acc

            def emit_back(h, sb, pt, acc, v_t, nsplit=1, shalves=1):
                """sums -> PV -> normalize -> store for (h, sb).

                shalves > 1 splits the whole back phase into s-column
                halves so the drain chain after the final PV matmul is
                half as long (used for the last pipeline block only).
                """
                s0 = sb * SBLK
                sw = SBLK // shalves
                for so in range(0, SBLK, sw):
                    sums_p = pss.tile([128, SBLK], F32, tag="sums")
                    for ti in range(KDVE, TC):
                        nc.tensor.matmul(sums_p[:, :sw], ones_m[:],
                                         pt[:, ti, so:so + sw],
                                         start=(ti == KDVE), stop=False)
                    nc.tensor.matmul(sums_p[:, :sw], ones_m[:],
                                     acc[:, 0, so:so + sw],
                                     start=False, stop=False)
                    nc.tensor.matmul(sums_p[:, :sw], ones_m[:],
                                     acc[:, 1, so:so + sw],
                                     start=False, stop=True)
                    ctx_p = psc.tile([128, SBLK], F32, tag="ctx")
                    for ti in range(TC):
                        nc.tensor.matmul(ctx_p[:, :sw], v_t[:, ti, :],
                                         pt[:, ti, so:so + sw],
                                         start=(ti == 0), stop=(ti == TC - 1))
                    w = sw // nsplit
                    for o in range(0, sw, w):
                        recip = sbmisc.tile([128, w], F32, tag="recip")
                        nc.vector.reciprocal_approx_fast(
                            recip[:], sums_p[:, o:o + w])
                        ctx_s = sbmisc.tile([128, w], BF16, tag="ctxs")
                        nc.vector.tensor_mul(ctx_s[:], ctx_p[:, o:o + w],
                                             recip[:])
                        nc.sync.dma_start(
                            out=ctxT_d[h, :, s0 + so + o:s0 + so + o + w],
                            in_=ctx_s[:])

            def body(_iv=None):
                # warm the PE clock (HAM) and the ACT exp table with dummy
                # ops that only depend on the memset, while the first DMAs
                # land
                warm_p = pss.tile([1, SBLK], F32, tag="sums")
                for _ in range(6):
                    nc.tensor.matmul(warm_p[:], ones_b[:], warm_src[:],
                                     start=True, stop=True)
                nc.scalar.activation(warm_e[:], warm_src[:, 0:16], EXP,
                                     scale=SCALE)

                pendings = []   # [(h, sb, pt, acc, v_t), ...]
                qkv = {}
                for h in range(HPC):
                    qT_t = sbqkv.tile([128, SQ], BF16, tag="qT")
                    kT_t = sbqkv.tile([128, SQ], BF16, tag="kT")
                    v_t = sbqkv.tile([128, TC, HN], BF16, tag="v")
                    v_r = v_d[h].rearrange("(c p) d -> p c d", p=128)
                    if h == 0:
                        # need-ordered loads, trigger-frugal: each
                        # dma_start costs ~0.6us of SP sequencer time
                        # (DIRECT2D), and consumers wait on whole-DMA
                        # semaphores — so the first-needed operands go
                        # as small pieces and the bulk as few large
                        # descriptor-efficient transfers.
                        for c in range(8):
                            nc.sync.dma_start(
                                out=kT_t[:, 256 * c:256 * (c + 1)],
                                in_=kT_d[h][:, 256 * c:256 * (c + 1)])
                        for c in range(2):
                            nc.sync.dma_start(
                                out=qT_t[:, 256 * c:256 * (c + 1)],
                                in_=qT_d[h][:, 256 * c:256 * (c + 1)])
                        for qtr in range(4):
                            nc.sync.dma_start(
                                out=keep_t[:, 0, 4 * qtr:4 * (qtr + 1), :],
                                in_=keep_d[:, 0, 4 * qtr:4 * (qtr + 1), :])
                        for c in range(1, NSB):
                            nc.sync.dma_start(
                                out=qT_t[:, SBLK * c:SBLK * (c + 1)],
                                in_=qT_d[h][:, SBLK * c:SBLK * (c + 1)])
                        nc.sync.dma_start(out=keep_t[:, 1], in_=keep_d[:, 1])
                        for half in range(2):
                            nc.sync.dma_start(
                                out=v_t[:, 8 * half:8 * (half + 1), :],
                                in_=v_r[:, 8 * half:8 * (half + 1), :])
                        nc.sync.dma_start(out=keep_t[:, 2], in_=keep_d[:, 2])
                        nc.sync.dma_start(out=keep_t[:, 3], in_=keep_d[:, 3])
                    else:
                        for half in range(2):
                            cols = slice(SQ // 2 * half, SQ // 2 * (half + 1))
                            nc.sync.dma_start(out=qT_t[:, cols],
                                              in_=qT_d[h][:, cols])
                            nc.sync.dma_start(out=kT_t[:, cols],
                                              in_=kT_d[h][:, cols])
                            nc.sync.dma_start(
                                out=v_t[:, 8 * half:8 * (half + 1), :],
                                in_=v_r[:, 8 * half:8 * (half + 1), :])
                    qkv[h] = (qT_t, kT_t, v_t)
                    last_head = h == HPC - 1
                    for sb in range(NSB):
                        # shallow out the pipeline before the final block
                        # so the drain tail after the last front is short
                        if last_head and sb == NSB - 1:
                            while len(pendings) > 1:
                                emit_back(*pendings.pop(0))
                        pt, acc = emit_front(h, sb, qT_t, kT_t)
                        pendings.append((h, sb, pt, acc, v_t))
                        if len(pendings) > DEPTH:
                            emit_back(*pendings.pop(0))
                for i, p in enumerate(pendings):
                    emit_back(*p, shalves=1 + i)

            if repeat == 1:
                body()
            else:
                with tc.For_i(0, repeat, 1):
                    body()
    nc.compile()
    return nc


def _get_nc(repeat=1):
    if repeat not in _CACHE:
        _CACHE[repeat] = _build(repeat)
    return _CACHE[repeat]


def _make_in_maps(query_layer, key_layer, value_layer, attention_mask):
    import ml_dtypes
    bf16 = ml_dtypes.bfloat16
    q = np.asarray(query_layer, dtype=np.float32)
    k = np.asarray(key_layer, dtype=np.float32)
    v = np.asarray(value_layer, dtype=np.float32)
    m = np.asarray(attention_mask)
    in_maps = []
    for c in range(NCORES):
        b = c // 4
        h0 = 4 * (c % 4)
        hs = slice(h0, h0 + HPC)
        qT = np.ascontiguousarray(
            q[:, b, hs, :].transpose(1, 2, 0)).astype(bf16)    # [4,hn,sq]
        kT = np.ascontiguousarray(
            k[:, b, hs, :].transpose(1, 2, 0)).astype(bf16)
        vv = np.ascontiguousarray(
            v[:, b, hs, :].transpose(1, 0, 2)).astype(bf16)    # [4,sq,hn]
        # keep swizzled to [p, sb, c, s']: keep2[p, sb, c, s'] =
        # keep[t = c*128 + p, s = sb*512 + s']
        keep_ts = (m[b, 0] == 0).T.astype(bf16)                # [t,s] bf16
        keepT = np.ascontiguousarray(
            keep_ts.reshape(16, 128, 4, 512).transpose(1, 2, 0, 3))
        in_maps.append({"qT": qT, "kT": kT, "v": vv, "keepT": keepT})
    return in_maps


def _run(nc, in_maps):
    from concourse.bass_utils import run_bass_kernel_spmd
    return run_bass_kernel_spmd(nc, in_maps, list(range(NCORES)))


def kernel(query_layer, key_layer, value_layer, attention_mask):
    in_maps = _make_in_maps(query_layer, key_layer, value_layer, attention_mask)
    res = _run(_get_nc(1), in_maps)
    out = np.empty((SQ, B, NH, HN), dtype=np.float32)
    for c in range(NCORES):
        b = c // 4
        h0 = 4 * (c % 4)
        ctxT = np.asarray(res.results[c]["ctxT"], dtype=np.float32)   # [4,hn,sq]
        out[:, b, h0:h0 + HPC, :] = ctxT.transpose(2, 0, 1)
    return out.reshape(SQ, B, NH * HN)
